# revision 20
# baseline (speedup 1.0000x reference)
"""Multi-head attention (N=2, L=2048, E=1024, H=16) on 8 TRN2 NeuronCores.

Sharding: DP2 x TP4 (Megatron-style).  Core c owns batch n = c//4 and
head-group hg = c%4 (4 heads = 256 embed dims).  It computes Q/K/V
projections only for its 4 heads but over ALL 2048 tokens of its batch,
full attention for those heads, and a *partial* output projection
against its 256 rows of Wo.T.  The host sums the 4 partials per batch
(the row-parallel reduce) -- zero redundant FLOPs on device: 8.6
GFLOP/core vs 15.0 for the batch x query-slice sharding.

The critical resource is the ScalarE (ACT) engine: 2048q x 2048k x 4
heads = 16.8M exps/core at 1 elem/lane/cycle @1.2GHz ~= 138us.  The
schedule keeps ACT saturated and hides all PE work in the ~860ns of
PE slack under each [128,1024] exp:

  - scores^T[k,q] per head pair via d=64 matmuls at partition offsets
    0/64 (two heads run concurrently in separate PE row groups).
  - V is augmented with a ones column; the 65-row ctx^T matmul then
    yields the softmax row sums in row 64 for free.
  - ctx PSUM is released by a single DVE copy to SBUF; the recip ->
    partition-broadcast -> mul normalization chain runs SBUF-side off
    the critical path (a 3-engine chain on the PSUM ring was measured
    to stall the in-order PE queue ~7us at every chunk boundary).
  - remaining projection quarters (deadline-ordered, "urgent" queue)
    and V blocks / ctx iterations / output-projection chunks ("normal"
    queue) are drip-fed between score matmuls by a cost-budgeted
    filler scheduler; ctx(kb) is queued one iteration late so the
    in-order PE queue never head-blocks on the exp it consumes.
  - all DRAM tensors are laid out exactly as their SBUF destination
    (partition-major), so every load is 128 contiguous descriptors.
  - dummy matmuls during the initial DMA wait warm the PE HAM clock
    gate (1.2 -> 2.4 GHz) before the first real projection.
  - bias is added by the DVE during the PSUM->SBUF output copy (bias
    input is zeroed for all but the hg==0 cores so the host sum adds
    it exactly once).

Layouts on device (per core):
  xT   [4][128p, 8e, 512]  : x[n].T, partition-major, column quarters
  w*T  [128p, 8e, 256] : W.T columns of this head group (wq pre-scaled)
  woT  [128p, 2, 1024] : Wo.T rows of this head group, 2 pair-blocks
  K^T/Q^T [2][128, 2048] : pair p rows = heads 2p (0-63), 2p+1 (64-127)
  V    [16][128l, 4h, 65] (col 64 = ones)
  p^T  [128k, 1024] bf16  (cols 0-511 head A, 512-1023 head B)
  ctx^T psum [65, 512] per (pair, head, qc); row 64 = softmax sums
  cN   [2][128, 2048] bf16 : normalized ctx^T = out-proj lhsT
  out  [16][128q, 1024] bf16 partial (summed across the 4 cores of
                               the batch on the host)
"""

import sys
from collections import deque
from contextlib import ExitStack

import numpy as np

if "/opt/trn_rl_repo" not in sys.path:
    sys.path.insert(0, "/opt/trn_rl_repo")

import ml_dtypes

import concourse.bass as bass
import concourse.mybir as mybir
import concourse.tile as tile
from concourse import bacc
from concourse.bass_utils import run_bass_kernel_spmd

EMBED = 1024
HEADS = 16
D = 64
N_BATCH = 2
L = 2048
P = 128
EB = 8            # 128-row blocks of the embed (contraction) dim
LB = 16           # 128-row blocks of the key/token dim
HC = 4            # heads per core
HGD = HC * D      # embed dims per head group (256)
NQC = 4           # query chunks
QCW = 512         # queries per chunk
NCORES = 8

BF16 = mybir.dt.bfloat16
F32 = mybir.dt.float32


def _build_bass():
    nc = bacc.Bacc()

    xTq = [
        nc.dram_tensor(f"xT{q}", (P, EB, QCW), BF16, kind="ExternalInput")
        for q in range(4)
    ]
    wqT = nc.dram_tensor("wqT", (P, EB, HGD), BF16, kind="ExternalInput")
    wkT = nc.dram_tensor("wkT", (P, EB, HGD), BF16, kind="ExternalInput")
    wvT = nc.dram_tensor("wvT", (P, EB, HGD), BF16, kind="ExternalInput")
    woT = nc.dram_tensor("woT", (P, 2, EMBED), BF16, kind="ExternalInput")
    bo = nc.dram_tensor("bo", (1, EMBED), BF16, kind="ExternalInput")
    out = nc.dram_tensor("out", (L // P, P, EMBED), BF16, kind="ExternalOutput")

    with tile.TileContext(nc) as tc, ExitStack() as ctx:
        _body(nc, tc, ctx, xTq, wqT, wkT, wvT, woT, bo, out)
    nc.compile()
    return nc


def _body(nc, tc, ctx, xTq, wqT, wkT, wvT, woT, bo, out):
    Exp = mybir.ActivationFunctionType.Exp

    persist = ctx.enter_context(tc.tile_pool(name="persist", bufs=1))
    KT = [persist.tile([P, L], BF16, tag=f"KT{p}", name=f"KT{p}") for p in range(2)]
    QT = [persist.tile([P, L], BF16, tag=f"QT{p}", name=f"QT{p}") for p in range(2)]
    V_sb = [
        persist.tile([P, HC, D + 1], BF16, tag=f"V{i}", name=f"V{i}") for i in range(LB)
    ]
    cN = [persist.tile([P, L], BF16, tag=f"cN{p}", name=f"cN{p}") for p in range(2)]
    bias128 = persist.tile([P, EMBED], BF16, tag="bias128", name="bias128")
    warm = persist.tile([P, QCW], BF16, tag="warm", name="warm")

    poolB = ctx.enter_context(tc.tile_pool(name="poolB", bufs=1))
    xq_sb = [poolB.tile([P, EB, QCW], BF16, tag=f"xq{q}", name=f"xq{q}") for q in range(4)]
    wk_sb = poolB.tile([P, EB, HGD], BF16, tag="wk", name="wk_sb")
    wq_sb = poolB.tile([P, EB, HGD], BF16, tag="wq", name="wq_sb")
    wv_sb = poolB.tile([P, EB, HGD], BF16, tag="wv", name="wv_sb")
    wo_sb = poolB.tile([P, 2, EMBED], BF16, tag="wo", name="wo_sb")
    bo_sb = poolB.tile([1, EMBED], BF16, tag="bo", name="bo_sb")

    # PSUM: psS 2x[P,1024] (4 banks) + psC 2x[P,512] (2) + psO 2x[P,512] (2)
    psS = ctx.enter_context(tc.tile_pool(name="psS", bufs=2, space="PSUM"))
    psC = ctx.enter_context(tc.tile_pool(name="psC", bufs=2, space="PSUM"))
    psO = ctx.enter_context(tc.tile_pool(name="psO", bufs=2, space="PSUM"))

    ptp = ctx.enter_context(tc.tile_pool(name="ptp", bufs=40))
    smp = ctx.enter_context(tc.tile_pool(name="smp", bufs=3))
    osb = ctx.enter_context(tc.tile_pool(name="osb", bufs=4))

    # ---- DMA (critical path first: per-queue FIFO means first-issued
    # transfers land first) + PE warm-up sized to flip the HAM clock
    # gate (needs ~3.4us of sustained PE busy) just before the real
    # projections start
    nc.sync.dma_start(out=wk_sb[:, :, 0:P], in_=wkT.ap()[:, :, 0:P])
    nc.sync.dma_start(out=wq_sb[:, :, 0:P], in_=wqT.ap()[:, :, 0:P])
    for q in range(4):
        nc.sync.dma_start(out=xq_sb[q], in_=xTq[q].ap())
        if q == 1:
            nc.sync.dma_start(out=wv_sb, in_=wvT.ap())
    nc.sync.dma_start(out=wk_sb[:, :, P:HGD], in_=wkT.ap()[:, :, P:HGD])
    nc.sync.dma_start(out=wq_sb[:, :, P:HGD], in_=wqT.ap()[:, :, P:HGD])
    nc.sync.dma_start(out=wo_sb, in_=woT.ap())
    nc.sync.dma_start(out=bo_sb, in_=bo.ap())
    nc.gpsimd.partition_broadcast(bias128, bo_sb)

    nc.vector.memset(warm, 0.0)
    for i in range(9):
        pw = psO.tile([P, QCW], F32, tag="o", name="warmps")
        nc.tensor.matmul(pw, warm[:, 0:P], warm, start=True, stop=True)

    # ---- emission helpers --------------------------------------------
    def kq_quarter(pr, qq, w_sb, pool, tag):
        """K^T or Q^T block pr, columns [qq*512, qq*512+512)."""
        ps = pool.tile([P, 1024] if pool is psS else [P, QCW], F32, tag=tag, name="kq")
        for e in range(EB):
            nc.tensor.matmul(
                ps[:, 0:QCW],
                w_sb[:, e, pr * P : (pr + 1) * P],
                xq_sb[qq][:, e, :],
                start=(e == 0),
                stop=(e == EB - 1),
            )
        tgt = (KT if w_sb is wk_sb else QT)[pr]
        nc.vector.tensor_copy(out=tgt[:, qq * QCW : (qq + 1) * QCW], in_=ps[:, 0:QCW])

    def v_block(lb):
        psv = psO.tile([P, QCW], F32, tag="o", name="psv")
        for e in range(EB):
            nc.tensor.matmul(
                psv[:, 0:HGD],
                xq_sb[lb // 4][:, e, (lb % 4) * P : (lb % 4 + 1) * P],
                wv_sb[:, e, :],
                start=(e == 0),
                stop=(e == EB - 1),
            )
        nc.vector.memset(V_sb[lb][:, :, D : D + 1], 1.0)
        nc.vector.tensor_copy(
            out=V_sb[lb][:, :, 0:D],
            in_=psv[:, 0:HGD].rearrange("p (h d) -> p h d", d=D),
        )

    cps = {}  # (pr, qc) -> [ctxA, ctxB] psum tiles

    def ctx_iter(pr, qc, kb, pt):
        if kb == 0:
            cps[(pr, qc)] = [
                psC.tile([P, QCW], F32, tag="ctx", name=f"c{pr}{qc}{hi}")
                for hi in range(2)
            ]
        for hi in range(2):
            nc.tensor.matmul(
                cps[(pr, qc)][hi][0 : D + 1, :],
                V_sb[kb][:, 2 * pr + hi, :],
                pt[:, hi * 512 : (hi + 1) * 512],
                start=(kb == 0),
                stop=(kb == LB - 1),
            )
        if kb == LB - 1:
            norm(pr, qc)

    def norm(pr, qc):
        """Free the ctx PSUM with one DVE copy; normalize SBUF-side."""
        for hi in range(2):
            cp = cps[(pr, qc)][hi]
            cu = smp.tile([D + 1, QCW], F32, tag="cu", name="cu", bufs=3)
            nc.vector.tensor_copy(out=cu, in_=cp[0 : D + 1, :])
            recip = smp.tile([1, QCW], F32, tag="recip", name="recip", bufs=3)
            nc.vector.reciprocal(out=recip, in_=cu[D : D + 1, :])
            bcs = smp.tile([D, QCW], F32, tag="bcs", name="bcs", bufs=3)
            nc.gpsimd.partition_broadcast(bcs, recip)
            nc.vector.tensor_mul(
                cN[pr][64 * hi : 64 * hi + 64, qc * QCW : (qc + 1) * QCW],
                cu[0:D, :],
                bcs,
            )
        del cps[(pr, qc)]

    def out_chunk(qt, c):
        pso = psO.tile([P, QCW], F32, tag="o", name="pso")
        for pr in range(2):
            nc.tensor.matmul(
                pso,
                cN[pr][:, qt * P : (qt + 1) * P],
                wo_sb[:, pr, c * 512 : (c + 1) * 512],
                start=(pr == 0),
                stop=(pr == 1),
            )
        ot = osb.tile([P, QCW], BF16, tag="ot", name="ot")
        nc.vector.tensor_add(ot, pso, bias128[:, c * 512 : (c + 1) * 512])
        nc.sync.dma_start(out=out[qt][:, c * 512 : (c + 1) * 512], in_=ot)

    # ---- three-priority filler scheduler -----------------------------
    # urgent: K/Q projection quarters (hard deadlines vs. the score
    #         stream).  ctxq: ctx iterations -- must stay within a few
    #         iterations of the exp stream or the psC/pt rings and the
    #         norm chain latency pile up into boundary stalls.  normal:
    #         V blocks and out-proj chunks (a full chunk of slack).
    urgent = deque()
    ctxq = deque()
    normal = deque()
    spent = [0.0]
    budget = [0.0]

    def run_fillers(extra_ns):
        # cap banked credit at ~2 iterations: a drained queue must not
        # accumulate budget that later dumps many ops into the in-order
        # PE queue at once (measured 6.5us PE+ACT stalls per boundary)
        budget[0] = min(budget[0] + extra_ns, spent[0] + 1720)
        while spent[0] < budget[0] and (urgent or ctxq or normal):
            q = urgent if urgent else (ctxq if ctxq else normal)
            c, fn = q.popleft()
            fn()
            spent[0] += c

    # ---- prologue: first K/Q quarters of pair 0 ----------------------
    kq_quarter(0, 0, wk_sb, psS, "s")
    kq_quarter(0, 0, wq_sb, psS, "s")

    for qq in range(1, 4):
        urgent.append((1750, lambda qq=qq: kq_quarter(0, qq, wk_sb, psO, "o")))
    urgent.append((1750, lambda: kq_quarter(0, 1, wq_sb, psO, "o")))
    # V blocks go in ctxq AHEAD of the ctx iterations that read them
    # (FIFO order = emission order = dependency correctness)
    for lb in range(LB):
        ctxq.append((950, lambda lb=lb: v_block(lb)))

    # ---- main: 2 pairs x 4 query chunks x 16 key blocks --------------
    for pr in range(2):
        for qc in range(NQC):
            if pr == 0 and qc == 1:
                for qq in range(2, 4):
                    urgent.append(
                        (1750, lambda qq=qq: kq_quarter(0, qq, wq_sb, psO, "o"))
                    )
            if pr == 0 and qc == 2:
                urgent.append((1750, lambda: kq_quarter(1, 0, wk_sb, psO, "o")))
                urgent.append((1750, lambda: kq_quarter(1, 0, wq_sb, psO, "o")))
            if pr == 1:
                if qc == 0:
                    for qq in range(1, 4):
                        urgent.append(
                            (1750, lambda qq=qq: kq_quarter(1, qq, wk_sb, psO, "o"))
                        )
                if qc < 3:
                    # Q^T quarter qc+1 must be *emitted* before chunk
                    # qc+1's score matmuls, so queue it one chunk early
                    urgent.append(
                        (1750, lambda qq=qc + 1: kq_quarter(1, qq, wq_sb, psO, "o"))
                    )
                if qc >= 1:
                    # out-proj for qc-1 (cN of both pairs ready by now)
                    for qt in range(4 * (qc - 1), 4 * qc):
                        for c in range(2):
                            normal.append(
                                (500, lambda qt=qt, c=c: out_chunk(qt, c))
                            )
            for kb in range(LB):
                pss = psS.tile([P, 1024], F32, tag="s", name="pss")
                for hi in range(2):
                    nc.tensor.matmul(
                        pss[:, hi * 512 : (hi + 1) * 512],
                        KT[pr][64 * hi : 64 * hi + 64, kb * P : (kb + 1) * P],
                        QT[pr][64 * hi : 64 * hi + 64, qc * QCW : (qc + 1) * QCW],
                        start=True,
                        stop=True,
                    )
                pt = ptp.tile([P, 1024], BF16, tag="pt", name="pt")
                nc.scalar.activation(out=pt, in_=pss, func=Exp)
                run_fillers(860)
                # queued after run_fillers: ctx(kb) pops at iteration
                # kb+1 at the earliest, so the in-order PE queue never
                # head-blocks on the exp it consumes
                ctxq.append(
                    (430, lambda pr=pr, qc=qc, kb=kb, pt=pt: ctx_iter(pr, qc, kb, pt))
                )

    # ---- tail --------------------------------------------------------
    while urgent or ctxq or normal:
        q = urgent if urgent else (ctxq if ctxq else normal)
        c, fn = q.popleft()
        fn()
    # last chunk's out-proj on the now-free score PSUM banks: 2-deep
    # [P,1024] ring so PE matmuls overlap the DVE bias-add copies
    for qt in range(12, 16):
        pso = psS.tile([P, 1024], F32, tag="s", name="psoT")
        for pr in range(2):
            for c in range(2):
                nc.tensor.matmul(
                    pso[:, c * 512 : (c + 1) * 512],
                    cN[pr][:, qt * P : (qt + 1) * P],
                    wo_sb[:, pr, c * 512 : (c + 1) * 512],
                    start=(pr == 0),
                    stop=(pr == 1),
                )
        ot = osb.tile([P, EMBED], BF16, tag="otw", name="otw")
        nc.vector.tensor_add(ot, pso, bias128)
        nc.sync.dma_start(out=out[qt], in_=ot)


_NC_CACHE = None


def _get_nc():
    global _NC_CACHE
    if _NC_CACHE is None:
        _NC_CACHE = _build_bass()
    return _NC_CACHE


def _make_in_maps(x, Wq, Wk, Wv, Wo, bo):
    bf = ml_dtypes.bfloat16
    xb = np.asarray(x, dtype=np.float32)
    scale = 1.0 / np.sqrt(np.float32(EMBED))
    wqT = np.ascontiguousarray(np.asarray(Wq, np.float32).T * scale)
    wkT = np.ascontiguousarray(np.asarray(Wk, np.float32).T)
    wvT = np.ascontiguousarray(np.asarray(Wv, np.float32).T)
    woT = np.ascontiguousarray(np.asarray(Wo, np.float32).T)
    bob = np.asarray(bo, np.float32).astype(bf).reshape(1, EMBED)
    bzero = np.zeros((1, EMBED), dtype=bf)

    def pmajor(w):  # [E, cols] -> [P, EB, cols] with partition-major rows
        return np.ascontiguousarray(
            w.reshape(EB, P, w.shape[1]).transpose(1, 0, 2)
        ).astype(bf)

    # x[n].T as [P, EB, L], split into column quarters
    xq = []
    for n in range(N_BATCH):
        xt = np.ascontiguousarray(xb[n].T).reshape(EB, P, L).transpose(1, 0, 2)
        xq.append(
            [
                np.ascontiguousarray(xt[:, :, q * QCW : (q + 1) * QCW]).astype(bf)
                for q in range(4)
            ]
        )

    in_maps = []
    for c in range(NCORES):
        n, hg = c // 4, c % 4
        hs = slice(hg * HGD, (hg + 1) * HGD)
        wos = np.ascontiguousarray(woT[hs, :]).reshape(2, P, EMBED).transpose(1, 0, 2)
        m = {
            "wqT": pmajor(np.ascontiguousarray(wqT[:, hs])),
            "wkT": pmajor(np.ascontiguousarray(wkT[:, hs])),
            "wvT": pmajor(np.ascontiguousarray(wvT[:, hs])),
            "woT": np.ascontiguousarray(wos).astype(bf),
            "bo": bob if hg == 0 else bzero,
        }
        for q in range(4):
            m[f"xT{q}"] = xq[n][q]
        in_maps.append(m)
    return in_maps


def _run(x, Wq, Wk, Wv, Wo, bo, trace=False):
    nc = _get_nc()
    in_maps = _make_in_maps(x, Wq, Wk, Wv, Wo, bo)
    res = run_bass_kernel_spmd(nc, in_maps, core_ids=list(range(NCORES)), trace=trace)
    full = np.zeros((N_BATCH, L, EMBED), np.float32)
    for c in range(NCORES):
        n = c // 4
        full[n] += res.results[c]["out"].reshape(L, EMBED).astype(np.float32)
    return full, res


def kernel(x, Wq, Wk, Wv, Wo, bo):
    full, _ = _run(x, Wq, Wk, Wv, Wo, bo, trace=False)
    return full


# revision 29
# speedup vs baseline: 1.1533x; 1.1533x over previous
"""Multi-head attention (N=2, L=2048, E=1024, H=16) on 8 TRN2 NeuronCores.

Sharding: DP2 x TP4 (Megatron-style).  Core c owns batch n = c//4 and
head-group hg = c%4 (4 heads = 256 embed dims).  It computes Q/K/V
projections only for its 4 heads but over ALL 2048 tokens of its batch,
full attention for those heads, and a *partial* output projection
against its 256 rows of Wo.T.  The host sums the 4 partials per batch
(the row-parallel reduce) -- zero redundant FLOPs on device: 8.6
GFLOP/core vs 15.0 for the batch x query-slice sharding.

The critical resource is the ScalarE (ACT) engine: 2048q x 2048k x 4
heads = 16.8M exps/core at 1 elem/lane/cycle @1.2GHz ~= 138us.  The
schedule keeps ACT saturated and hides all PE work in the ~860ns of
PE slack under each [128,1024] exp:

  - scores^T[k,q] per head pair via d=64 matmuls at partition offsets
    0/64 (two heads run concurrently in separate PE row groups).
  - V is augmented with a ones column; the 65-row ctx^T matmul then
    yields the softmax row sums in row 64 for free.
  - ctx PSUM is released by a single DVE copy to SBUF; the recip ->
    partition-broadcast -> mul normalization chain runs SBUF-side off
    the critical path (a 3-engine chain on the PSUM ring was measured
    to stall the in-order PE queue ~7us at every chunk boundary).
  - remaining projection quarters (deadline-ordered, "urgent" queue)
    and V blocks / ctx iterations / output-projection chunks ("normal"
    queue) are drip-fed between score matmuls by a cost-budgeted
    filler scheduler; ctx(kb) is queued one iteration late so the
    in-order PE queue never head-blocks on the exp it consumes.
  - all DRAM tensors are laid out exactly as their SBUF destination
    (partition-major), so every load is 128 contiguous descriptors.
  - dummy matmuls during the initial DMA wait warm the PE HAM clock
    gate (1.2 -> 2.4 GHz) before the first real projection.
  - bias is added by the DVE during the PSUM->SBUF output copy (bias
    input is zeroed for all but the hg==0 cores so the host sum adds
    it exactly once).

Layouts on device (per core):
  xT   [4][128p, 8e, 512]  : x[n].T, partition-major, column quarters
  w*T  [128p, 8e, 256] : W.T columns of this head group (wq pre-scaled)
  woT  [128p, 2, 1024] : Wo.T rows of this head group, 2 pair-blocks
  K^T/Q^T [2][128, 2048] : pair p rows = heads 2p (0-63), 2p+1 (64-127)
  V    [16][128l, 4h, 65] (col 64 = ones)
  p^T  [128k, 1024] bf16  (cols 0-511 head A, 512-1023 head B)
  ctx^T psum [65, 512] per (pair, head, qc); row 64 = softmax sums
  cN   [2][128, 2048] bf16 : normalized ctx^T = out-proj lhsT
  out  [16][128q, 1024] bf16 partial (summed across the 4 cores of
                               the batch on the host)
"""

import sys
from collections import deque
from contextlib import ExitStack

import numpy as np

if "/opt/trn_rl_repo" not in sys.path:
    sys.path.insert(0, "/opt/trn_rl_repo")

import ml_dtypes

import concourse.bass as bass
import concourse.mybir as mybir
import concourse.tile as tile
from concourse import bacc
from concourse.bass_utils import run_bass_kernel_spmd

EMBED = 1024
HEADS = 16
D = 64
N_BATCH = 2
L = 2048
P = 128
EB = 8            # 128-row blocks of the embed (contraction) dim
LB = 16           # 128-row blocks of the key/token dim
HC = 4            # heads per core
HGD = HC * D      # embed dims per head group (256)
NQC = 4           # query chunks
QCW = 512         # queries per chunk
NCORES = 8

BF16 = mybir.dt.bfloat16
F32 = mybir.dt.float32


def _build_bass():
    nc = bacc.Bacc()

    xTq = [
        nc.dram_tensor(f"xT{q}", (P, EB, QCW), BF16, kind="ExternalInput")
        for q in range(4)
    ]
    wqT = nc.dram_tensor("wqT", (P, EB, HGD), BF16, kind="ExternalInput")
    wkT = nc.dram_tensor("wkT", (P, EB, HGD), BF16, kind="ExternalInput")
    wvT = nc.dram_tensor("wvT", (P, EB, HGD), BF16, kind="ExternalInput")
    woT = nc.dram_tensor("woT", (P, 2, EMBED), BF16, kind="ExternalInput")
    bo = nc.dram_tensor("bo", (1, EMBED), BF16, kind="ExternalInput")
    out = nc.dram_tensor("out", (L // P, P, EMBED), BF16, kind="ExternalOutput")

    with tile.TileContext(nc) as tc, ExitStack() as ctx:
        _body(nc, tc, ctx, xTq, wqT, wkT, wvT, woT, bo, out)
    nc.compile()
    return nc


def _body(nc, tc, ctx, xTq, wqT, wkT, wvT, woT, bo, out):
    Exp = mybir.ActivationFunctionType.Exp

    persist = ctx.enter_context(tc.tile_pool(name="persist", bufs=1))
    KT = [persist.tile([P, L], BF16, tag=f"KT{p}", name=f"KT{p}") for p in range(2)]
    QT = [persist.tile([P, L], BF16, tag=f"QT{p}", name=f"QT{p}") for p in range(2)]
    V_sb = [
        persist.tile([P, HC, D + 1], BF16, tag=f"V{i}", name=f"V{i}") for i in range(LB)
    ]
    cN = [persist.tile([P, L], BF16, tag=f"cN{p}", name=f"cN{p}") for p in range(2)]
    bias128 = persist.tile([P, EMBED], BF16, tag="bias128", name="bias128")
    warm = persist.tile([P, QCW], BF16, tag="warm", name="warm")

    poolB = ctx.enter_context(tc.tile_pool(name="poolB", bufs=1))
    xq_sb = [poolB.tile([P, EB, QCW], BF16, tag=f"xq{q}", name=f"xq{q}") for q in range(4)]
    wk_sb = poolB.tile([P, EB, HGD], BF16, tag="wk", name="wk_sb")
    wq_sb = poolB.tile([P, EB, HGD], BF16, tag="wq", name="wq_sb")
    wv_sb = poolB.tile([P, EB, HGD], BF16, tag="wv", name="wv_sb")
    wo_sb = poolB.tile([P, 2, EMBED], BF16, tag="wo", name="wo_sb")
    bo_sb = poolB.tile([1, EMBED], BF16, tag="bo", name="bo_sb")

    # PSUM: psS 2x[P,1024] (4 banks) + psC 2x[P,512] (2) + psO 2x[P,512] (2)
    psS = ctx.enter_context(tc.tile_pool(name="psS", bufs=2, space="PSUM"))
    psC = ctx.enter_context(tc.tile_pool(name="psC", bufs=2, space="PSUM"))
    psO = ctx.enter_context(tc.tile_pool(name="psO", bufs=2, space="PSUM"))

    ptp = ctx.enter_context(tc.tile_pool(name="ptp", bufs=40))
    smp = ctx.enter_context(tc.tile_pool(name="smp", bufs=3))
    osb = ctx.enter_context(tc.tile_pool(name="osb", bufs=4))

    # ---- DMA (critical path first: per-queue FIFO means first-issued
    # transfers land first) + PE warm-up sized to flip the HAM clock
    # gate (needs ~3.4us of sustained PE busy) just before the real
    # projections start
    nc.sync.dma_start(out=wk_sb[:, :, 0:P], in_=wkT.ap()[:, :, 0:P])
    nc.sync.dma_start(out=wq_sb[:, :, 0:P], in_=wqT.ap()[:, :, 0:P])
    for e in range(EB):
        nc.sync.dma_start(out=xq_sb[0][:, e, :], in_=xTq[0].ap()[:, e, :])
    for q in range(1, 4):
        nc.sync.dma_start(out=xq_sb[q], in_=xTq[q].ap())
        if q == 1:
            nc.sync.dma_start(out=wv_sb, in_=wvT.ap())
    nc.sync.dma_start(out=wk_sb[:, :, P:HGD], in_=wkT.ap()[:, :, P:HGD])
    nc.sync.dma_start(out=wq_sb[:, :, P:HGD], in_=wqT.ap()[:, :, P:HGD])
    nc.sync.dma_start(out=wo_sb, in_=woT.ap())
    nc.sync.dma_start(out=bo_sb, in_=bo.ap())
    nc.gpsimd.partition_broadcast(bias128, bo_sb)

    nc.vector.memset(warm, 0.0)
    for i in range(9):
        pw = psO.tile([P, QCW], F32, tag="o", name="warmps")
        nc.tensor.matmul(pw, warm[:, 0:P], warm, start=True, stop=True)

    # ---- emission helpers --------------------------------------------
    def kq_quarter(pr, qq, w_sb, pool, tag):
        """K^T or Q^T block pr, columns [qq*512, qq*512+512)."""
        ps = pool.tile([P, 1024] if pool is psS else [P, QCW], F32, tag=tag, name="kq")
        for e in range(EB):
            nc.tensor.matmul(
                ps[:, 0:QCW],
                w_sb[:, e, pr * P : (pr + 1) * P],
                xq_sb[qq][:, e, :],
                start=(e == 0),
                stop=(e == EB - 1),
            )
        tgt = (KT if w_sb is wk_sb else QT)[pr]
        nc.vector.tensor_copy(out=tgt[:, qq * QCW : (qq + 1) * QCW], in_=ps[:, 0:QCW])

    def v_block(lb):
        psv = psO.tile([P, QCW], F32, tag="o", name="psv")
        for e in range(EB):
            nc.tensor.matmul(
                psv[:, 0:HGD],
                xq_sb[lb // 4][:, e, (lb % 4) * P : (lb % 4 + 1) * P],
                wv_sb[:, e, :],
                start=(e == 0),
                stop=(e == EB - 1),
            )
        nc.vector.memset(V_sb[lb][:, :, D : D + 1], 1.0)
        nc.vector.tensor_copy(
            out=V_sb[lb][:, :, 0:D],
            in_=psv[:, 0:HGD].rearrange("p (h d) -> p h d", d=D),
        )

    cps = {}  # (pr, qc) -> [ctxA, ctxB] psum tiles

    def ctx_iter(pr, qc, kb, pt):
        if kb == 0:
            cps[(pr, qc)] = [
                psC.tile([P, QCW], F32, tag="ctx", name=f"c{pr}{qc}{hi}")
                for hi in range(2)
            ]
        for hi in range(2):
            nc.tensor.matmul(
                cps[(pr, qc)][hi][0 : D + 1, :],
                V_sb[kb][:, 2 * pr + hi, :],
                pt[:, hi * 512 : (hi + 1) * 512],
                start=(kb == 0),
                stop=(kb == LB - 1),
            )
        if kb == LB - 1:
            norm(pr, qc)

    def norm(pr, qc):
        """Free the ctx PSUM with the two DVE copies FIRST (they gate the
        next chunk's ctx via the psC ring), then normalize SBUF-side with
        the ~5x cheaper Newton-Raphson reciprocal (~51 ULP -- far beyond
        what a softmax denominator needs; sums are ~2e3, no edge cases)."""
        cus = []
        for hi in range(2):
            cu = smp.tile([D + 1, QCW], F32, tag="cu", name="cu", bufs=3)
            nc.vector.tensor_copy(out=cu, in_=cps[(pr, qc)][hi][0 : D + 1, :])
            cus.append(cu)
        for hi in range(2):
            cu = cus[hi]
            recip = smp.tile([1, QCW], F32, tag="recip", name="recip", bufs=3)
            nc.vector.reciprocal(out=recip, in_=cu[D : D + 1, :])
            bcs = smp.tile([D, QCW], F32, tag="bcs", name="bcs", bufs=3)
            nc.gpsimd.partition_broadcast(bcs, recip)
            nc.vector.tensor_mul(
                cN[pr][64 * hi : 64 * hi + 64, qc * QCW : (qc + 1) * QCW],
                cu[0:D, :],
                bcs,
            )
        del cps[(pr, qc)]
        if pr == 1 and qc < 3:
            # cN for chunk qc is now complete for both pairs (pair-0's
            # norm ran a whole pair earlier) -- out-proj may be queued.
            # Appending here (not at the chunk boundary) guarantees the
            # out-proj is EMITTED after this norm under any pop order.
            # qc==3 is handled by the tail's wide out-proj on psS.
            for qt in range(4 * qc, 4 * qc + 4):
                for c in range(2):
                    normal.append((500, lambda qt=qt, c=c: out_chunk(qt, c)))

    def out_chunk(qt, c):
        pso = psO.tile([P, QCW], F32, tag="o", name="pso")
        for pr in range(2):
            nc.tensor.matmul(
                pso,
                cN[pr][:, qt * P : (qt + 1) * P],
                wo_sb[:, pr, c * 512 : (c + 1) * 512],
                start=(pr == 0),
                stop=(pr == 1),
            )
        ot = osb.tile([P, QCW], BF16, tag="ot", name="ot")
        nc.vector.tensor_add(ot, pso, bias128[:, c * 512 : (c + 1) * 512])
        nc.sync.dma_start(out=out[qt][:, c * 512 : (c + 1) * 512], in_=ot)

    # ---- three-priority filler scheduler -----------------------------
    # urgent: K/Q projection quarters (hard deadlines vs. the score
    #         stream).  ctxq: ctx iterations -- must stay within a few
    #         iterations of the exp stream or the psC/pt rings and the
    #         norm chain latency pile up into boundary stalls.  normal:
    #         V blocks and out-proj chunks (a full chunk of slack).
    urgent = deque()
    ctxq = deque()
    normal = deque()
    spent = [0.0]
    budget = [0.0]
    rr = [0]

    def run_fillers(extra_ns):
        # cap banked credit at ~2 iterations: a drained queue must not
        # accumulate budget that later dumps many ops into the in-order
        # PE queue at once (measured 6.5us PE+ACT stalls per boundary)
        budget[0] = min(budget[0] + extra_ns, spent[0] + 1720)
        while spent[0] < budget[0] and (urgent or ctxq or normal):
            if urgent:
                q = urgent
            else:
                # round-robin ctx against V/out-proj so neither builds
                # a deep backlog (a locked 1-chunk ctx lag turns into a
                # boundary stall and a long tail)
                rr[0] ^= 1
                first, second = (ctxq, normal) if rr[0] else (normal, ctxq)
                q = first if first else second
            c, fn = q.popleft()
            fn()
            spent[0] += c

    # ---- prologue: first K/Q quarters of pair 0 ----------------------
    kq_quarter(0, 0, wk_sb, psS, "s")
    kq_quarter(0, 0, wq_sb, psS, "s")

    for qq in range(1, 4):
        urgent.append((1750, lambda qq=qq: kq_quarter(0, qq, wk_sb, psO, "o")))
    urgent.append((1750, lambda: kq_quarter(0, 1, wq_sb, psO, "o")))
    # V blocks go in ctxq AHEAD of the ctx iterations that read them
    # (FIFO order = emission order = dependency correctness)
    for lb in range(LB):
        ctxq.append((950, lambda lb=lb: v_block(lb)))

    # ---- main: 2 pairs x 4 query chunks x 16 key blocks --------------
    for pr in range(2):
        for qc in range(NQC):
            if pr == 0 and qc == 1:
                for qq in range(2, 4):
                    urgent.append(
                        (1750, lambda qq=qq: kq_quarter(0, qq, wq_sb, psO, "o"))
                    )
            if pr == 0 and qc == 2:
                urgent.append((1750, lambda: kq_quarter(1, 0, wk_sb, psO, "o")))
                urgent.append((1750, lambda: kq_quarter(1, 0, wq_sb, psO, "o")))
            if pr == 1:
                if qc == 0:
                    for qq in range(1, 4):
                        urgent.append(
                            (1750, lambda qq=qq: kq_quarter(1, qq, wk_sb, psO, "o"))
                        )
                if qc < 3:
                    # Q^T quarter qc+1 must be *emitted* before chunk
                    # qc+1's score matmuls, so queue it one chunk early
                    urgent.append(
                        (1750, lambda qq=qc + 1: kq_quarter(1, qq, wq_sb, psO, "o"))
                    )

            for kb in range(LB):
                pss = psS.tile([P, 1024], F32, tag="s", name="pss")
                for hi in range(2):
                    nc.tensor.matmul(
                        pss[:, hi * 512 : (hi + 1) * 512],
                        KT[pr][64 * hi : 64 * hi + 64, kb * P : (kb + 1) * P],
                        QT[pr][64 * hi : 64 * hi + 64, qc * QCW : (qc + 1) * QCW],
                        start=True,
                        stop=True,
                    )
                pt = ptp.tile([P, 1024], BF16, tag="pt", name="pt")
                nc.scalar.activation(out=pt, in_=pss, func=Exp)
                run_fillers(860)
                # queued after run_fillers: ctx(kb) pops at iteration
                # kb+1 at the earliest, so the in-order PE queue never
                # head-blocks on the exp it consumes
                ctxq.append(
                    (430, lambda pr=pr, qc=qc, kb=kb, pt=pt: ctx_iter(pr, qc, kb, pt))
                )

    # ---- tail --------------------------------------------------------
    while urgent or ctxq or normal:
        q = urgent if urgent else (ctxq if ctxq else normal)
        c, fn = q.popleft()
        fn()
    # last chunk's out-proj on the now-free score PSUM banks: 2-deep
    # [P,1024] ring so PE matmuls overlap the DVE bias-add copies
    for qt in range(12, 16):
        pso = psS.tile([P, 1024], F32, tag="s", name="psoT")
        for pr in range(2):
            for c in range(2):
                nc.tensor.matmul(
                    pso[:, c * 512 : (c + 1) * 512],
                    cN[pr][:, qt * P : (qt + 1) * P],
                    wo_sb[:, pr, c * 512 : (c + 1) * 512],
                    start=(pr == 0),
                    stop=(pr == 1),
                )
        ot = osb.tile([P, EMBED], BF16, tag="otw", name="otw")
        nc.vector.tensor_add(ot, pso, bias128)
        nc.sync.dma_start(out=out[qt], in_=ot)


_NC_CACHE = None


def _get_nc():
    global _NC_CACHE
    if _NC_CACHE is None:
        _NC_CACHE = _build_bass()
    return _NC_CACHE


def _make_in_maps(x, Wq, Wk, Wv, Wo, bo):
    bf = ml_dtypes.bfloat16
    xb = np.asarray(x, dtype=np.float32)
    scale = 1.0 / np.sqrt(np.float32(EMBED))
    wqT = np.ascontiguousarray(np.asarray(Wq, np.float32).T * scale)
    wkT = np.ascontiguousarray(np.asarray(Wk, np.float32).T)
    wvT = np.ascontiguousarray(np.asarray(Wv, np.float32).T)
    woT = np.ascontiguousarray(np.asarray(Wo, np.float32).T)
    bob = np.asarray(bo, np.float32).astype(bf).reshape(1, EMBED)
    bzero = np.zeros((1, EMBED), dtype=bf)

    def pmajor(w):  # [E, cols] -> [P, EB, cols] with partition-major rows
        return np.ascontiguousarray(
            w.reshape(EB, P, w.shape[1]).transpose(1, 0, 2)
        ).astype(bf)

    # x[n].T as [P, EB, L], split into column quarters
    xq = []
    for n in range(N_BATCH):
        xt = np.ascontiguousarray(xb[n].T).reshape(EB, P, L).transpose(1, 0, 2)
        xq.append(
            [
                np.ascontiguousarray(xt[:, :, q * QCW : (q + 1) * QCW]).astype(bf)
                for q in range(4)
            ]
        )

    in_maps = []
    for c in range(NCORES):
        n, hg = c // 4, c % 4
        hs = slice(hg * HGD, (hg + 1) * HGD)
        wos = np.ascontiguousarray(woT[hs, :]).reshape(2, P, EMBED).transpose(1, 0, 2)
        m = {
            "wqT": pmajor(np.ascontiguousarray(wqT[:, hs])),
            "wkT": pmajor(np.ascontiguousarray(wkT[:, hs])),
            "wvT": pmajor(np.ascontiguousarray(wvT[:, hs])),
            "woT": np.ascontiguousarray(wos).astype(bf),
            "bo": bob if hg == 0 else bzero,
        }
        for q in range(4):
            m[f"xT{q}"] = xq[n][q]
        in_maps.append(m)
    return in_maps


def _run(x, Wq, Wk, Wv, Wo, bo, trace=False):
    nc = _get_nc()
    in_maps = _make_in_maps(x, Wq, Wk, Wv, Wo, bo)
    res = run_bass_kernel_spmd(nc, in_maps, core_ids=list(range(NCORES)), trace=trace)
    full = np.zeros((N_BATCH, L, EMBED), np.float32)
    for c in range(NCORES):
        n = c // 4
        full[n] += res.results[c]["out"].reshape(L, EMBED).astype(np.float32)
    return full, res


def kernel(x, Wq, Wk, Wv, Wo, bo):
    full, _ = _run(x, Wq, Wk, Wv, Wo, bo, trace=False)
    return full
